# revision 1
# baseline (speedup 1.0000x reference)
"""AdaLayerNorm (ragged gather_csr + LayerNorm) Trainium2 Bass kernel.

Runs SPMD on 8 NeuronCores, data-parallel over the node dimension: each core
gets a contiguous 16384-row shard of `feat`, replicated affine weights, and
its segment end-offsets clipped to the local row range, so the gather_csr
expansion stays device-local (per the sharding hint).

Per-core graph, HBM-roofline oriented:
- Chunked 2 MB DMAs in a p-major row layout: each of the 128 partitions holds
  8 consecutive rows, so every DMA descriptor moves one contiguous 16 KB span.
- LayerNorm stats via bn_stats (VectorE); alpha/beta small-ops on GpSimd;
  normalize on ScalarE as Identity(x*alpha + beta) with per-partition
  scale/bias vectors.
- The per-node segment broadcast g[seg(i), :] is computed on-device as ONE
  accumulating TensorE matmul per 128-row tile against a 0/1 comparison
  matrix A[s, i] = [row_i < end_s] (built on VectorE via is_lt against the
  per-core clipped offsets) and telescoped row differences
  dg[s] = g[s] - g[s+1]:  sum_{s: end_s > row} dg[s] == g[seg(row)] exactly.
  dg is fp16 (the only surviving rounding, ~1.6e-3 relative).
- Final out = normed * g_tile on VectorE, one instruction per chunk reading
  all 8 PSUM banks, in place, then streamed out.
- Iteration emission is software-pipelined (recip/beta at iteration head,
  stats split around the apply phase) to keep every engine FIFO busy.
"""

import sys

sys.path.insert(0, "/opt/trn_rl_repo")

import os

import numpy as np

import concourse.bacc as bacc
import concourse.mybir as mybir
import concourse.tile as tile
from concourse.bass_utils import run_bass_kernel_spmd
from concourse.tile_rust import add_dep_helper

dt = mybir.dt
AF = mybir.ActivationFunctionType
ALU = mybir.AluOpType

EPS = 1e-5
P = 128
N_CORES = 8
N, D_FULL, S_FULL = 131072, 512, 64
ROWS = N // N_CORES


def _install_profshim():
    """Best-effort NTFF profiling hook for trace runs (optional)."""
    try:
        import types

        import antenv

        if getattr(antenv, "axon_hooks", None) is not None:
            return
        sys.path.insert(0, "/root/.axon_site/trn_agent_boot")
        import trn_boot

        hook = trn_boot._ntff_profile_via_ctypes("/opt/axon/libaxon_pjrt.so")
        m = types.ModuleType("antenv.axon_hooks")
        state = {"hook": hook}
        m.set_axon_ntff_profile_hook = lambda h: state.__setitem__("hook", h)
        m.get_axon_ntff_profile_hook = lambda: state["hook"]
        sys.modules["antenv.axon_hooks"] = m
        antenv.axon_hooks = m
    except Exception:
        pass


def build_kernel(rows=16384, D=512, S=64, chunk_tiles=8, tt_batch=4, skew=1, prefetch=2, bufs_in=4, bufs_nrm=4, bufs_sgn=None, bufs_sts=None):
    bufs_sgn = bufs_sgn if bufs_sgn is not None else skew + 3
    bufs_sts = bufs_sts if bufs_sts is not None else skew + 3
    """Build the per-core Bass graph. All 8 cores run this same graph."""
    J = chunk_tiles
    chunk_rows = P * J
    nchunks = rows // chunk_rows
    assert rows % chunk_rows == 0
    assert J % tt_batch == 0
    WD = D
    KCH = WD // P

    nc = bacc.Bacc("TRN2", target_bir_lowering=False, debug=False)
    feat = nc.declare_dram_parameter("feat", [rows, D], dt.float32, isOutput=False)
    gfT = nc.declare_dram_parameter("gfT", [WD, S], dt.float32, isOutput=False)
    WT = nc.declare_dram_parameter("WT", [WD, D], dt.float32, isOutput=False)
    bb = nc.declare_dram_parameter("b", [1, D], dt.float32, isOutput=False)
    hi = nc.declare_dram_parameter("hi", [S, 1], dt.float32, isOutput=False)
    out = nc.declare_dram_parameter("out", [rows, D], dt.float32, isOutput=True)

    with tile.TileContext(nc) as tc:
        with (
            tc.tile_pool(name="const", bufs=1) as cst,
            tc.tile_pool(name="inb", bufs=bufs_in) as in_pool,
            tc.tile_pool(name="nrm", bufs=bufs_nrm) as nrm_pool,
            tc.tile_pool(name="sgn", bufs=bufs_sgn) as sgn_pool,
            tc.tile_pool(name="sts", bufs=bufs_sts) as sts_pool,
            tc.tile_pool(name="ps", bufs=(2 if tt_batch <= 4 else 1), space="PSUM") as ps_pool,
        ):
            # First feat chunks queue ahead of all init DMAs (startup latency)
            feat_v = feat.ap().rearrange("(c p j) d -> c p j d", p=P, j=J)
            out_v = out.ap().rearrange("(c p j) d -> c p j d", p=P, j=J)
            ints = {}

            def dma_in(c):
                ints[c] = in_pool.tile(
                    [P, J, D], dt.float32, tag="int", name=f"int{c}"
                )
                nc.sync.dma_start(ints[c][:], feat_v[c])

            for c0 in range(min(prefetch, nchunks)):
                dma_in(c0)

            # ---------------- init: affine g = gf @ W.T + b ----------------
            wt_sb = cst.tile([P, KCH, D], dt.float32)
            nc.sync.dma_start(wt_sb[:], WT.ap().rearrange("(k p) d -> p k d", p=P))
            gft_sb = cst.tile([P, KCH, S], dt.float32)
            nc.sync.dma_start(gft_sb[:], gfT.ap().rearrange("(k p) s -> p k s", p=P))
            b_sb = cst.tile([1, D], dt.float32)
            nc.sync.dma_start(b_sb[:], bb.ap())
            hi_sb = cst.tile([S, 1], dt.float32)
            nc.sync.dma_start(hi_sb[:], hi.ap())

            ones1 = cst.tile([1, S], dt.float32)
            nc.vector.memset(ones1[:], 1.0)

            psg = ps_pool.tile([S, D], dt.float32, tag="ps")
            for k in range(KCH):
                nc.tensor.matmul(
                    psg[:],
                    gft_sb[:, k, :],
                    wt_sb[:, k, :],
                    start=(k == 0),
                    stop=False,
                )
            nc.tensor.matmul(psg[:], ones1[:], b_sb[:], start=False, stop=True)
            # telescoped differences: dg[s] = g[s] - g[s+1]  (dg[63] = g[63]);
            # sum_{s >= seg} dg[s] telescopes exactly to g[seg]
            gf32 = cst.tile([S, D], dt.float32)
            nc.scalar.activation(gf32[:], psg[:], AF.Copy)
            gsh = cst.tile([S, D], dt.float32)
            nc.vector.memset(gsh[:], 0.0)
            nc.sync.dma_start(gsh[0 : S - 1, :], gf32[1:S, :])
            dg16 = cst.tile([S, D], dt.float16)
            nc.vector.tensor_tensor(dg16[:], gf32[:], gsh[:], ALU.subtract)

            # ------------ init: iota + per-chunk comparison biases ------------
            # p-major layout: flat column k = j*P + i holds row value j + J*i,
            # so sub-tile slice [:, j*P:(j+1)*P] column i maps to psum
            # partition i = local row i*J + j.
            iota = cst.tile([S, chunk_rows], dt.float32)
            nc.gpsimd.iota(
                iota[:],
                pattern=[[1, J], [J, P]],
                base=0,
                channel_multiplier=0,
                allow_small_or_imprecise_dtypes=True,
            )
            iotc = cst.tile([S, nchunks], dt.float32)
            nc.gpsimd.iota(
                iotc[:],
                pattern=[[1, nchunks]],
                base=0,
                channel_multiplier=0,
                allow_small_or_imprecise_dtypes=True,
            )
            # off_hi[s, c] = hi_s - c*chunk_rows (is_lt thresholds per chunk)
            off_hi = cst.tile([S, nchunks], dt.float32)
            nc.vector.tensor_scalar(
                off_hi[:],
                iotc[:],
                -float(chunk_rows),
                hi_sb[:],
                op0=ALU.mult,
                op1=ALU.add,
            )

            # ---------------- main loop (software-pipelined, skew 1) ----------------
            # Emission order per engine IS execution order (engines are FIFO).
            # Emitting stats(c) before apply(c-1) lets each engine run chunk
            # c's stats while other engines finish chunk c-1's apply.
            def stats_a(c):
                int_ = ints[c]
                sh = sgn_pool.tile([S, chunk_rows], dt.float16, tag="sh")
                nc.vector.tensor_scalar(
                    sh[:], iota[:], off_hi[:, c : c + 1], None, op0=ALU.is_lt
                )
                st6 = sts_pool.tile([P, J, 6], dt.float32, tag="st6")
                last_bn = None
                for j in range(J):
                    last_bn = nc.vector.bn_stats(st6[:, j, :], int_[:, j, :])
                return (sh, st6, last_bn)

            def stats_b(c, staged, after_ident=None):
                sh, st6, _ = staged
                m_e = st6[:, :, 1]
                cv_e = st6[:, :, 2]
                m_o = st6[:, :, 4]
                cv_o = st6[:, :, 5]

                msum = sts_pool.tile([P, J], dt.float32, tag="msum")
                nc.gpsimd.tensor_tensor(msum[:], m_e, m_o, ALU.add)
                mdif = sts_pool.tile([P, J], dt.float32, tag="mdif")
                nc.gpsimd.tensor_tensor(mdif[:], m_e, m_o, ALU.subtract)
                cvs = sts_pool.tile([P, J], dt.float32, tag="cvs")
                nc.gpsimd.tensor_tensor(cvs[:], cv_e, cv_o, ALU.add)
                msq = sts_pool.tile([P, J], dt.float32, tag="msq")
                sq_inst = nc.scalar.activation(msq[:], mdif[:], AF.Square, scale=0.5)
                if after_ident is not None:
                    add_dep_helper(
                        sq_inst.ins,
                        after_ident.ins,
                        sync=False,
                        reason="keep stats-tail ACT ops after prior chunk's identities",
                    )
                v = sts_pool.tile([P, J], dt.float32, tag="v")
                nc.gpsimd.tensor_scalar(
                    v[:], cvs[:], 1.0 / D, EPS, op0=ALU.mult, op1=ALU.add
                )
                nc.gpsimd.tensor_tensor(v[:], v[:], msq[:], ALU.add)
                sdev = sts_pool.tile([P, J], dt.float32, tag="sdev")
                sqrt_inst = nc.scalar.activation(sdev[:], v[:], AF.Sqrt)
                if after_ident is not None:
                    add_dep_helper(
                        sqrt_inst.ins,
                        after_ident.ins,
                        sync=False,
                        reason="keep stats-tail ACT ops after prior chunk's identities",
                    )
                negm = sts_pool.tile([P, J], dt.float32, tag="negm")
                nc.gpsimd.tensor_scalar(negm[:], msum[:], -0.5, None, op0=ALU.mult)
                return (sh, sdev, negm)

            def alphabeta(staged):
                sh, sdev, negm = staged
                alpha = sts_pool.tile([P, J], dt.float32, tag="alpha")
                nc.vector.reciprocal(alpha[:], sdev[:])
                beta = sts_pool.tile([P, J], dt.float32, tag="beta")
                nc.gpsimd.tensor_tensor(beta[:], negm[:], alpha[:], ALU.mult)
                return (sh, alpha, beta)

            def apply_phase(c, staged, next_bn=None):
                sh, alpha, beta = staged
                int_ = ints.pop(c)
                nrm = nrm_pool.tile([P, J, D], dt.float32, tag="nrm")
                last_ident = None
                for h in range(J // tt_batch):
                    ps = ps_pool.tile([P, tt_batch, D], dt.float32, tag="ps")
                    for jj in range(tt_batch):
                        j = h * tt_batch + jj
                        last_ident = nc.scalar.activation(
                            nrm[:, j, :],
                            int_[:, j, :],
                            AF.Identity,
                            bias=beta[:, j : j + 1],
                            scale=alpha[:, j : j + 1],
                        )
                        nc.tensor.matmul(
                            ps[:, jj, :],
                            sh[:, j * P : (j + 1) * P],
                            dg16[:],
                            start=True,
                            stop=True,
                        )
                    lohi = slice(h * tt_batch, (h + 1) * tt_batch)
                    nc.vector.tensor_tensor(
                        nrm[:, lohi, :], nrm[:, lohi, :], ps[:], ALU.mult
                    )
                nc.sync.dma_start(out_v[c], nrm[:])
                return last_ident

            staged = {}
            for c in range(nchunks):
                ab = alphabeta(staged.pop(c - skew)) if c >= skew else None
                if c + prefetch < nchunks:
                    dma_in(c + prefetch)
                part_a = stats_a(c)
                last_ident = None
                if ab is not None:
                    last_ident = apply_phase(c - skew, ab)
                staged[c] = stats_b(c, part_a, last_ident)
            for c in range(nchunks - skew, nchunks):
                apply_phase(c, alphabeta(staged.pop(c)))

    nc.compile()
    return nc


def make_in_maps(feat, global_feat, offset, W, b, n_cores=N_CORES):
    """Shard the full inputs into per-core in_maps."""
    N, D = feat.shape
    S = offset.shape[0]
    rows = N // n_cores
    feat = np.asarray(feat, dtype=np.float32)
    offset = np.asarray(offset, dtype=np.int64)
    gfT = np.ascontiguousarray(np.asarray(global_feat, dtype=np.float32).T)
    WT = np.ascontiguousarray(np.asarray(W, dtype=np.float32).T)
    b_ = np.asarray(b, dtype=np.float32).reshape(1, D)
    ends = offset
    starts = np.concatenate([[0], offset[:-1]])
    in_maps = []
    for c in range(n_cores):
        base = c * rows
        hi_c = np.clip(ends - base, 0, rows).astype(np.float32).reshape(S, 1)
        in_maps.append(
            {
                "feat": np.ascontiguousarray(feat[base : base + rows]),
                "gfT": gfT,
                "WT": WT,
                "b": b_,
                "hi": hi_c,
            }
        )
    return in_maps

_NC_CACHE = {}

last_exec_time_ns = None


def kernel(feat, global_feat, offset, W, b):
    """Full inputs in, full output out. Shards across 8 NeuronCores."""
    global last_exec_time_ns
    if "nc" not in _NC_CACHE:
        _NC_CACHE["nc"] = build_kernel(
            rows=ROWS, chunk_tiles=8, tt_batch=8, skew=1, prefetch=4, bufs_in=7, bufs_nrm=4
        )
    nc = _NC_CACHE["nc"]
    in_maps = make_in_maps(feat, global_feat, offset, W, b, n_cores=N_CORES)
    kwargs = {}
    if os.environ.get("ADALN_TRACE") == "1":
        _install_profshim()
        import tempfile

        kwargs = {"trace": True, "tmpdir": tempfile.mkdtemp(prefix="adaln_prof_")}
    res = run_bass_kernel_spmd(nc, in_maps, core_ids=list(range(N_CORES)), **kwargs)
    last_exec_time_ns = res.exec_time_ns
    return np.concatenate([res.results[i]["out"] for i in range(N_CORES)], axis=0)

